# revision 16
# baseline (speedup 1.0000x reference)
"""Trainium2 Bass kernel computing out = x * exp(diagonal).

x: (8192, 4096) float32, diagonal: (4096,) float32.
Data-parallel across 8 NeuronCores: each core handles 1024 rows of x;
the 4096-float diagonal is replicated to every core.

Per-core program (pure streaming; the 16 SDMA engines aggregate
~435 GB/s and bound the kernel, so 32 MiB of x in+out traffic floors
at ~78 us; everything else must hide under that):

  1. diagonal loads as a [1, 4096] tile via one tiny HWDGE DMA issued
     on the SP queue BEFORE the x loads (per-engine rings are FIFO, so
     it completes first), ACT computes exp in place.
  2. Partition-broadcast WITHOUT any DMA: TensorE outer-product
     ones[1,128]^T @ expd[1,4096] -> PSUM [128, 4096] (8 matmuls, one
     per 512-float PSUM bank). Zero HBM/fabric bytes. A 1-element DVE
     copy observes the last matmul so the muls below carry exactly one
     wait (their own load DMA).
  3. x streams through 8 fresh [128, 4096] SBUF tiles (no slot reuse
     => no WAR waits): HWDGE load on SP -> in-place DVE multiply with
     operand b read from PSUM -> HWDGE store on ACT. The LAST row
     block is split into four [128, 1024] column-quarters so the
     closing load-sem -> mul -> store-drain chain shrinks from
     ~11 us to ~4 us (the kernel end is store-drain-bound).
"""

import numpy as np

BATCH, FEAT = 8192, 4096
N_CORES = 8
ROWS = BATCH // N_CORES   # 1024 rows per core
P = 128                   # SBUF partitions
N_TILES = ROWS // P       # 8 tiles of [128, 4096] per core
PSUM_BANK = 512           # fp32 elems per PSUM bank (2 KiB)

_CACHE = {}


def build_nc(rows=ROWS, feat=FEAT):
    import concourse.bacc as bacc
    import concourse.mybir as mybir
    from concourse import tile

    # Bacc (not plain Bass): its compile() pass splits multi-sem waits into
    # EventSemaphore chains -- TRN2 instructions carry at most one wait.
    nc = bacc.Bacc("TRN2", target_bir_lowering=False, debug=False)
    x = nc.dram_tensor("x", (rows, feat), mybir.dt.float32, kind="ExternalInput").ap()
    d = nc.dram_tensor("d", (feat,), mybir.dt.float32, kind="ExternalInput").ap()
    out = nc.dram_tensor(
        "out", (rows, feat), mybir.dt.float32, kind="ExternalOutput"
    ).ap()

    n_tiles = rows // P
    x_t = x.rearrange("(s p) m -> s p m", p=P)
    o_t = out.rearrange("(s p) m -> s p m", p=P)
    d_row = d.rearrange("(r c) -> r c", r=1)

    with tile.TileContext(nc) as tc:
        with (
            tc.tile_pool(name="const", bufs=1) as cpool,
            tc.tile_pool(name="psum", bufs=1, space="PSUM") as ppool,
            tc.tile_pool(name="io", bufs=n_tiles - 1) as iopool,
            tc.tile_pool(name="ioq", bufs=4) as qpool,
        ):
            d1 = cpool.tile([1, feat], mybir.dt.float32)
            ones = cpool.tile([1, P], mybir.dt.float32)
            expd = ppool.tile([P, feat], mybir.dt.float32)

            # diagonal -> [1, feat]: first DMA on the SP queue, so its
            # descriptors drain before x tile 0's on every engine ring.
            nc.sync.dma_start(d1[:], d_row)
            nc.scalar.activation(d1[:], d1[:], mybir.ActivationFunctionType.Exp)
            nc.vector.memset(ones[:], 1.0)
            # Broadcast across partitions: ones^T @ expd, one matmul per
            # PSUM bank (512 fp32).
            for b in range(feat // PSUM_BANK):
                sl = slice(b * PSUM_BANK, (b + 1) * PSUM_BANK)
                nc.tensor.matmul(
                    expd[:, sl], ones[:], d1[:, sl], start=True, stop=True
                )
            # DVE observer: absorbs the wait on the matmuls so the muls
            # below carry exactly one wait (their own load DMA).
            scratch = cpool.tile([1, 1], mybir.dt.float32)
            nc.vector.tensor_copy(scratch[:], expd[0:1, 0:1])

            tiles = []
            for i in range(n_tiles - 1):
                t = iopool.tile([P, feat], mybir.dt.float32)
                nc.sync.dma_start(t[:], x_t[i])
                tiles.append(t)
            # Last row block as four column-quarters (partition dim stays
            # 128 -- sub-128-partition DMAs wreck descriptor balance).
            QC = feat // 4
            qtiles = []
            for q in range(4):
                tq = qpool.tile([P, QC], mybir.dt.float32)
                nc.sync.dma_start(tq[:], x_t[n_tiles - 1][:, q * QC : (q + 1) * QC])
                qtiles.append(tq)

            for i, t in enumerate(tiles):
                nc.vector.tensor_mul(t[:], t[:], expd[:])
                nc.scalar.dma_start(o_t[i], t[:])
            for q, tq in enumerate(qtiles):
                sl = slice(q * QC, (q + 1) * QC)
                nc.vector.tensor_mul(tq[:], tq[:], expd[:, sl])
                nc.scalar.dma_start(o_t[n_tiles - 1][:, sl], tq[:])
    nc.finalize()
    return nc


def kernel(x, diagonal):
    from concourse.bass_utils import run_bass_kernel_spmd

    if "nc" not in _CACHE:
        _CACHE["nc"] = build_nc()
    nc = _CACHE["nc"]

    x = np.ascontiguousarray(x, dtype=np.float32)
    d = np.ascontiguousarray(diagonal, dtype=np.float32)
    in_maps = [{"x": x[c * ROWS : (c + 1) * ROWS], "d": d} for c in range(N_CORES)]
    res = run_bass_kernel_spmd(nc, in_maps, core_ids=list(range(N_CORES)))
    _CACHE["last_res"] = res
    return np.concatenate([r["out"] for r in res.results], axis=0)
